# revision 10
# baseline (speedup 1.0000x reference)
"""CrossAttention (B=1, S=4096, H=8, DH=40) on 8 Trainium2 NeuronCores.

Sharding: tensor-parallel over the 8 heads — core h computes head h's full
attention plus its partial output projection; the host sums the 8 partials
and adds the bias.

Per-core dataflow (all matmuls in float32r — TF32-like fast fp32 path):
  qT/kT  [40, 4096] = Wq_h @ x.T            (PE, K=320 in 3 chunks)
  vT     [40, 4096] = Wv_h @ x.T            (PE)  -> PE-transposed to
  v'     [4096, 65]: cols 0..39 = v, col 64 = 1.0 (row-sum trick)
  ST     [128j, 512i] = k_j @ q_i.T         (PE, K=40)
  PT     = exp(ST / sqrt(40))               (ScalarE, PSUM->SBUF, f32r out)
  O'.T/r [65, 512i] accumulate v'_j.T @ PT_j over j  (PE, K=128)
  rec    = 1/r (DVE reciprocal), broadcast over partitions via K=1 matmul
  oT     [40, 512i] = O'.T * rec            (DVE)
  Y_s    [128, 320] = oT_s.T @ Wo_h.T       (PE, K=40) -> DMA out
"""

import numpy as np

import concourse.bass as bass
import concourse.mybir as mybir
from concourse import bass_utils, masks
from concourse.tile import TileContext

S = 4096
D = 320
H = 8
DH = 40
N_CORES = 8
CHUNK = 512               # i-chunk width (PSUM bank, fp32)
N_CHUNKS = S // CHUNK     # 8
JT = S // 128             # 32 j-tiles
VW = 65                   # stationary width of v': 40 v cols + junk + ones col
SCALE = float(DH) ** -0.5

F32 = mybir.dt.float32
F32R = mybir.dt.float32r
EXP = mybir.ActivationFunctionType.Exp

_COMPILED = {}


def _split_sync_waits(nc, max_waits=1):
    """This walrus build rejects instructions with more than one sync wait.
    Spill the excess onto same-engine nops placed just before the
    instruction (engine streams execute in program order, so all waits are
    satisfied before the instruction issues)."""
    for f in nc.m.functions:
        for bb in f.blocks:
            out = []
            changed = False
            for inst in bb.instructions:
                si = inst.sync_info
                if si is not None and si.on_wait and len(si.on_wait) > max_waits:
                    waits = list(si.on_wait)
                    for i in range(max_waits, len(waits), max_waits):
                        nop = mybir.InstNoOp(
                            name=nc.get_next_instruction_name(),
                            engine=inst.engine,
                            bass_nofuse=True,
                            sync_info=mybir.SyncInfo(
                                on_wait=waits[i:i + max_waits], on_update=[]),
                        )
                        out.append(nop)
                    inst.sync_info = mybir.SyncInfo(
                        on_wait=waits[:max_waits],
                        on_update=list(si.on_update or []))
                    changed = True
                out.append(inst)
            if changed:
                bb.instructions = out


def _build(s=None, split=True):
    import os
    SKIP = set((os.environ.get('KSKIP') or '').split(','))
    s = s or S
    n_chunks = s // CHUNK
    jt = s // 128
    nc = bass.Bass('TRN2', target_bir_lowering=False, debug=False)

    xT_d = nc.dram_tensor('xT', [D, s], F32R, kind='ExternalInput').ap()
    wq_d = nc.dram_tensor('wq', [D, DH], F32R, kind='ExternalInput').ap()
    wk_d = nc.dram_tensor('wk', [D, DH], F32R, kind='ExternalInput').ap()
    wv_d = nc.dram_tensor('wv', [D, DH], F32R, kind='ExternalInput').ap()
    woT_d = nc.dram_tensor('woT', [DH, D], F32R, kind='ExternalInput').ap()
    out_d = nc.dram_tensor('out', [s, D], F32, kind='ExternalOutput').ap()

    KCH = (128, 128, 64)  # K chunks of D=320

    with TileContext(nc) as tc:
        with tc.tile_pool(name='const', bufs=1) as cpool, \
             tc.tile_pool(name='big', bufs=1) as big, \
             tc.tile_pool(name='pt', bufs=4) as ptp, \
             tc.tile_pool(name='work', bufs=3) as wk, \
             tc.tile_pool(name='ps_st', bufs=1, space='PSUM') as ps_st, \
             tc.tile_pool(name='ps_small', bufs=2, space='PSUM') as ps_small, \
             tc.tile_pool(name='ps_av', bufs=2, space='PSUM') as ps_av:

            # ---- constants & inputs ----
            ident = cpool.tile([128, 128], F32, tag='ident')
            masks.make_identity(nc, ident[:, :])

            xt0 = big.tile([128, s], F32R, tag='xt0')
            xt1 = big.tile([128, s], F32R, tag='xt1')
            xt2 = big.tile([64, s], F32R, tag='xt2')
            xts = (xt0, xt1, xt2)
            nc.sync.dma_start(xt0[:, :], xT_d[0:128, :])
            nc.sync.dma_start(xt1[:, :], xT_d[128:256, :])
            nc.sync.dma_start(xt2[:, :], xT_d[256:320, :])

            w_sbs = {}
            for nm, dram in (('wq', wq_d), ('wk', wk_d), ('wv', wv_d)):
                t = cpool.tile([128, 3 * DH], F32R, tag=nm)
                for c, kk in enumerate(KCH):
                    o = sum(KCH[:c])
                    nc.sync.dma_start(t[0:kk, c * DH:(c + 1) * DH],
                                      dram[o:o + kk, :])
                w_sbs[nm] = t
            woT_sb = cpool.tile([DH, D], F32R, tag='woT')
            nc.sync.dma_start(woT_sb[:, :], woT_d)

            qT = big.tile([DH, s], F32R, tag='qT')
            kT = big.tile([DH, s], F32R, tag='kT')
            # v plus 24 zero rows and a ones row 64: transposing gives v'
            # tiles whose col 64 is 1.0 (the row-sum column).
            vT = big.tile([VW, s], F32, tag='vT')
            vsb = big.tile([128, jt * VW], F32R, tag='vsb')
            oT = big.tile([DH, s], F32R, tag='oT')

            # zero rows 32..63 first (32-aligned base); phase 1 then
            # overwrites rows 0..39 with v, leaving 40..63 zero
            nc.vector.memset(vT[32:VW - 1, :], 0.0)
            nc.vector.memset(vT[VW - 1:VW, :], 1.0)

            # ones (f32r) at partition 64 for the rec broadcast matmul
            ones64 = cpool.tile([65, DH], F32R, tag='ones64')
            ones64_f = cpool.tile([65, DH], F32, tag='ones64f')
            nc.vector.memset(ones64_f[64:65, :], 1.0)
            nc.vector.tensor_copy(ones64[64:65, :], ones64_f[64:65, :])

            # ---- phase 1: qT / kT / vT ----
            def proj(dst, w_sb, c):
                ps = ps_small.tile([DH, CHUNK], F32, tag='small')
                for ci, kk in enumerate(KCH):
                    nc.tensor.matmul(
                        ps[:, :],
                        w_sb[0:kk, ci * DH:(ci + 1) * DH],
                        xts[ci][0:kk, c * CHUNK:(c + 1) * CHUNK],
                        start=(ci == 0), stop=(ci == 2))
                nc.vector.tensor_copy(dst[:, c * CHUNK:(c + 1) * CHUNK],
                                      ps[:, :])

            for c in range(n_chunks):
                proj(kT, w_sbs['wk'], c)
            for c in range(n_chunks):
                proj(vT[0:DH, :], w_sbs['wv'], c)
            for c in range(n_chunks):
                proj(qT, w_sbs['wq'], c)

            # ---- phase 2: transpose vT -> v' tiles ----
            for j in range(jt):
                tp = ps_small.tile([128, VW], F32, tag='small')
                nc.tensor.transpose(tp[:, :], vT[:, j * 128:(j + 1) * 128],
                                    ident[0:VW, 0:VW])
                nc.vector.tensor_copy(vsb[:, j * VW:(j + 1) * VW], tp[:, :])

            # ---- phase 3+4: attention per i-chunk ----
            GJ = 4  # j-tiles per exp group (4 PSUM banks)
            for c in range(n_chunks):
                av = ps_av.tile([VW, CHUNK], F32, tag='av')
                pts = []
                for g in range(jt // GJ):
                    st = ps_st.tile([128, GJ * CHUNK], F32, tag='st4')
                    for j2 in range(GJ):
                        j = g * GJ + j2
                        nc.tensor.matmul(
                            st[:, j2 * CHUNK:(j2 + 1) * CHUNK],
                            kT[:, j * 128:(j + 1) * 128],
                            qT[:, c * CHUNK:(c + 1) * CHUNK],
                            start=True, stop=True)
                    pt = ptp.tile([128, GJ * CHUNK], F32R, tag='pt')
                    if 'exp' in SKIP:
                        nc.vector.tensor_copy(pt[:, :], st[:, :])
                    else:
                        nc.scalar.activation(pt[:, :], st[:, :], EXP, scale=SCALE)
                    pts.append(pt)
                av_iters = [(g, j2) for g in range(jt // GJ) for j2 in range(GJ)]
                if 'avlong' in SKIP:
                    av_iters = av_iters[:1]
                n_av = len(av_iters)
                for idx, (g, j2) in enumerate(av_iters):
                    j = g * GJ + j2
                    nc.tensor.matmul(
                        av[:, :],
                        vsb[:, j * VW:(j + 1) * VW],
                        pts[g][:, j2 * CHUNK:(j2 + 1) * CHUNK],
                        start=(idx == 0), stop=(idx == n_av - 1))

                rec = wk.tile([65, CHUNK], F32R, tag='rec')
                with nc.allow_low_precision(reason='f32r rounding of 1/r'):
                    nc.vector.reciprocal(rec[64:65, :], av[64:65, :])
                rbc = ps_small.tile([DH, CHUNK], F32, tag='small')
                nc.tensor.matmul(rbc[:, :], ones64[64:65, :],
                                 rec[64:65, :], start=True, stop=True)
                rbc_sb = wk.tile([DH, CHUNK], F32, tag='rbc_sb')
                nc.vector.tensor_copy(rbc_sb[:, :], rbc[:, :])
                nc.vector.tensor_tensor(
                    out=oT[:, c * CHUNK:(c + 1) * CHUNK],
                    in0=av[0:DH, :], in1=rbc_sb[:, :],
                    op=mybir.AluOpType.mult)

                # output projection for the 4 s-tiles of this chunk
                for s2 in range(CHUNK // 128 if 'wo' not in SKIP else 0):
                    s = c * (CHUNK // 128) + s2
                    yp = ps_small.tile([128, D], F32, tag='small')
                    nc.tensor.matmul(yp[:, :],
                                     oT[:, s * 128:(s + 1) * 128],
                                     woT_sb[:, :], start=True, stop=True)
                    if 'wocopy' not in SKIP:
                        ysb = wk.tile([128, D], F32, tag='ysb')
                        nc.vector.tensor_copy(ysb[:, :], yp[:, :])
                        if 'wodma' not in SKIP:
                            nc.sync.dma_start(
                                out_d[s * 128:(s + 1) * 128, :], ysb[:, :])

    if split:
        _split_sync_waits(nc)
    return nc


def kernel(x, Wq, Wk, Wv, Wo, bo):
    x = np.asarray(x, dtype=np.float32)
    Wq = np.asarray(Wq, dtype=np.float32)
    Wk = np.asarray(Wk, dtype=np.float32)
    Wv = np.asarray(Wv, dtype=np.float32)
    Wo = np.asarray(Wo, dtype=np.float32)
    bo = np.asarray(bo, dtype=np.float32)

    if 'nc' not in _COMPILED:
        _COMPILED['nc'] = _build()
    nc = _COMPILED['nc']

    xT = np.ascontiguousarray(x.reshape(S, D).T)
    in_maps = []
    for h in range(N_CORES):
        sl = slice(h * DH, (h + 1) * DH)
        in_maps.append({
            'xT': xT,
            'wq': np.ascontiguousarray(Wq[sl, :].T),
            'wk': np.ascontiguousarray(Wk[sl, :].T),
            'wv': np.ascontiguousarray(Wv[sl, :].T),
            'woT': np.ascontiguousarray(Wo[:, sl].T),
        })

    import os
    trace = bool(os.environ.get('BASS_KERNEL_TRACE'))
    res = bass_utils.run_bass_kernel_spmd(
        nc, in_maps, core_ids=list(range(N_CORES)), trace=trace,
        tmpdir=os.environ.get('BASS_KERNEL_TRACE_DIR') or None)
    _COMPILED['last_res'] = res

    acc = res.results[0]['out'].astype(np.float32).copy()
    for h in range(1, N_CORES):
        acc += res.results[h]['out']
    acc += bo[None, :]
    return acc.reshape(1, S, D)


# revision 13
# speedup vs baseline: 1.0392x; 1.0392x over previous
"""CrossAttention (B=1, S=4096, H=8, DH=40) on 8 Trainium2 NeuronCores.

Sharding: tensor-parallel over the 8 heads — core h computes head h's full
attention plus its partial output projection; the host sums the 8 partials
and adds the bias.

Per-core dataflow (attention matmuls in bf16; fp32 accumulation in PSUM;
softmax renormalization cancels most of the bf16 rounding of P):
  qT/kT  [40, 4096] = Wq_h @ x.T            (PE, K=320 in 3 chunks)
  vT     [40, 4096] = Wv_h @ x.T            (PE)  -> PE-transposed to
  v'     [4096, 65]: cols 0..39 = v, col 64 = 1.0 (row-sum trick)
  ST     [128j, 512i] = k_j @ q_i.T         (PE, K=40)
  PT     = exp(ST / sqrt(40))               (ScalarE, PSUM->SBUF, bf16 out)
  O'.T|r [65, 512i] accumulate v'_j.T @ PT_j over j  (PE, K=128)
  rec    = 1/r (DVE), broadcast over partitions via K=1 matmul (fp32)
  oT     [40, 512i] = O'.T * rec            (DVE, bf16 out)
  Y_s    [128, 320] = oT_s.T @ Wo_h.T       (PE, K=40) -> DMA out
"""

import os

import ml_dtypes
import numpy as np

import concourse.bass as bass
import concourse.mybir as mybir
from concourse import bass_utils, masks
from concourse.tile import TileContext

S = 4096
D = 320
H = 8
DH = 40
N_CORES = 8
CHUNK = 512               # i-chunk width (one fp32 PSUM bank)
VW = 65                   # v' stationary width: 40 v cols, junk, ones col 64
GJ = 4                    # j-tiles per exp group (4 PSUM banks)
SCALE = float(DH) ** -0.5

F32 = mybir.dt.float32
F32R = mybir.dt.float32r
BF16 = mybir.dt.bfloat16
EXP = mybir.ActivationFunctionType.Exp

_COMPILED = {}


def _split_sync_waits(nc, max_waits=1):
    """This walrus build rejects instructions with more than one sync wait.
    Spill the excess onto same-engine nops placed just before the
    instruction (engine streams execute in program order, so all waits are
    satisfied before the instruction issues)."""
    for f in nc.m.functions:
        for bb in f.blocks:
            out = []
            changed = False
            for inst in bb.instructions:
                si = inst.sync_info
                if si is not None and si.on_wait and len(si.on_wait) > max_waits:
                    waits = list(si.on_wait)
                    for i in range(max_waits, len(waits), max_waits):
                        nop = mybir.InstNoOp(
                            name=nc.get_next_instruction_name(),
                            engine=inst.engine,
                            bass_nofuse=True,
                            sync_info=mybir.SyncInfo(
                                on_wait=waits[i:i + max_waits], on_update=[]),
                        )
                        out.append(nop)
                    inst.sync_info = mybir.SyncInfo(
                        on_wait=waits[:max_waits],
                        on_update=list(si.on_update or []))
                    changed = True
                out.append(inst)
            if changed:
                bb.instructions = out


def _build(s=None, split=True):
    s = s or S
    n_chunks = s // CHUNK
    jt = s // 128
    nc = bass.Bass('TRN2', target_bir_lowering=False, debug=False)

    xT_d = nc.dram_tensor('xT', [D, s], BF16, kind='ExternalInput').ap()
    wq_d = nc.dram_tensor('wq', [D, DH], BF16, kind='ExternalInput').ap()
    wk_d = nc.dram_tensor('wk', [D, DH], BF16, kind='ExternalInput').ap()
    wv_d = nc.dram_tensor('wv', [D, DH], BF16, kind='ExternalInput').ap()
    woT_d = nc.dram_tensor('woT', [DH, D], BF16, kind='ExternalInput').ap()
    out_d = nc.dram_tensor('out', [s, D], F32, kind='ExternalOutput').ap()

    KCH = (128, 128, 64)  # K chunks of D=320

    with TileContext(nc) as tc:
        with tc.tile_pool(name='const', bufs=1) as cpool, \
             tc.tile_pool(name='big', bufs=1) as big, \
             tc.tile_pool(name='pt', bufs=10) as ptp, \
             tc.tile_pool(name='work', bufs=3) as wkp, \
             tc.tile_pool(name='ps_st', bufs=1, space='PSUM') as ps_st, \
             tc.tile_pool(name='ps_small', bufs=2, space='PSUM') as ps_small, \
             tc.tile_pool(name='ps_av', bufs=2, space='PSUM') as ps_av:

            # ---- constants & inputs ----
            ident = cpool.tile([128, 128], F32, tag='ident')
            masks.make_identity(nc, ident[:, :])

            xt0 = big.tile([128, s], BF16, tag='xt0')
            xt1 = big.tile([128, s], BF16, tag='xt1')
            xt2 = big.tile([64, s], BF16, tag='xt2')
            xts = (xt0, xt1, xt2)
            # chunked loads so phase 1 starts before the whole xT lands
            for c in range(n_chunks):
                cs = slice(c * CHUNK, (c + 1) * CHUNK)
                nc.sync.dma_start(xt0[:, cs], xT_d[0:128, cs])
                nc.sync.dma_start(xt1[:, cs], xT_d[128:256, cs])
                nc.sync.dma_start(xt2[:, cs], xT_d[256:320, cs])

            w_sbs = {}
            for nm, dram in (('wq', wq_d), ('wk', wk_d), ('wv', wv_d)):
                t = cpool.tile([128, 3 * DH], BF16, tag=nm)
                for c, kk in enumerate(KCH):
                    o = sum(KCH[:c])
                    nc.sync.dma_start(t[0:kk, c * DH:(c + 1) * DH],
                                      dram[o:o + kk, :])
                w_sbs[nm] = t
            woT_sb = cpool.tile([DH, D], BF16, tag='woT')
            nc.sync.dma_start(woT_sb[:, :], woT_d)

            qT = big.tile([DH, s], BF16, tag='qT')
            kT = big.tile([DH, s], BF16, tag='kT')
            # v plus zero rows 40..63 and ones row 64: transposing yields v'
            # tiles whose col 64 is 1.0 (the row-sum column).
            vT = big.tile([VW, s], F32, tag='vT')
            vsb = big.tile([128, jt * VW], BF16, tag='vsb')
            oT = big.tile([DH, s], BF16, tag='oT')

            # zero rows 32..63 first (32-aligned base); phase 1 then
            # overwrites rows 0..39 with v, leaving 40..63 zero
            nc.vector.memset(vT[32:VW - 1, :], 0.0)
            nc.vector.memset(vT[VW - 1:VW, :], 1.0)

            # ones at partition 64 for the rec broadcast matmul (fp32)
            ones64 = cpool.tile([65, DH], F32, tag='ones64')
            nc.vector.memset(ones64[64:65, :], 1.0)

            # ---- phase 1 helpers ----
            def proj(dst, w_sb, c):
                ps = ps_small.tile([DH, CHUNK], F32, tag='small')
                for ci, kk in enumerate(KCH):
                    nc.tensor.matmul(
                        ps[:, :],
                        w_sb[0:kk, ci * DH:(ci + 1) * DH],
                        xts[ci][0:kk, c * CHUNK:(c + 1) * CHUNK],
                        start=(ci == 0), stop=(ci == 2))
                nc.vector.tensor_copy(dst[:, c * CHUNK:(c + 1) * CHUNK],
                                      ps[:, :])

            def transpose_v(j):
                tp = ps_small.tile([128, VW], F32, tag='small')
                nc.tensor.transpose(tp[:, :], vT[:, j * 128:(j + 1) * 128],
                                    ident[0:VW, 0:VW])
                nc.vector.tensor_copy(vsb[:, j * VW:(j + 1) * VW], tp[:, :])

            # k first, then q(0), so chunk 0's ST/exp can start while v is
            # still being built
            for c in range(n_chunks):
                proj(kT, w_sbs['wk'], c)
            proj(qT, w_sbs['wq'], 0)

            # ---- main loop over i-chunks ----
            for c in range(n_chunks):
                pts = []
                for g in range(jt // GJ):
                    st = ps_st.tile([128, GJ * CHUNK], F32, tag='st4')
                    for j2 in range(GJ):
                        j = g * GJ + j2
                        nc.tensor.matmul(
                            st[:, j2 * CHUNK:(j2 + 1) * CHUNK],
                            kT[:, j * 128:(j + 1) * 128],
                            qT[:, c * CHUNK:(c + 1) * CHUNK],
                            start=True, stop=True)
                    pt = ptp.tile([128, GJ * CHUNK], BF16, tag='pt')
                    nc.scalar.activation(pt[:, :], st[:, :], EXP, scale=SCALE)
                    pts.append(pt)

                if c == 0:
                    for cc in range(n_chunks):
                        proj(vT[0:DH, :], w_sbs['wv'], cc)
                    for j in range(jt):
                        transpose_v(j)
                if c + 1 < n_chunks:
                    proj(qT, w_sbs['wq'], c + 1)

                av = ps_av.tile([VW, CHUNK], F32, tag='av')
                for j in range(jt):
                    nc.tensor.matmul(
                        av[:, :],
                        vsb[:, j * VW:(j + 1) * VW],
                        pts[j // GJ][:, (j % GJ) * CHUNK:(j % GJ + 1) * CHUNK],
                        start=(j == 0), stop=(j == jt - 1))

                rec = wkp.tile([65, CHUNK], F32, tag='rec')
                nc.vector.reciprocal(rec[64:65, :], av[64:65, :])
                rbc = ps_small.tile([DH, CHUNK], F32, tag='small')
                nc.tensor.matmul(rbc[:, :], ones64[64:65, :],
                                 rec[64:65, :], start=True, stop=True)
                rbc_sb = wkp.tile([DH, CHUNK], F32, tag='rbc_sb')
                nc.vector.tensor_copy(rbc_sb[:, :], rbc[:, :])
                nc.vector.tensor_tensor(
                    out=oT[:, c * CHUNK:(c + 1) * CHUNK],
                    in0=av[0:DH, :], in1=rbc_sb[:, :],
                    op=mybir.AluOpType.mult)

                # output projection for the 4 s-tiles of this chunk
                for s2 in range(CHUNK // 128):
                    st_i = c * (CHUNK // 128) + s2
                    yp = ps_small.tile([128, D], F32, tag='small')
                    nc.tensor.matmul(yp[:, :],
                                     oT[:, st_i * 128:(st_i + 1) * 128],
                                     woT_sb[:, :], start=True, stop=True)
                    ysb = wkp.tile([128, D], F32, tag='ysb')
                    nc.vector.tensor_copy(ysb[:, :], yp[:, :])
                    nc.sync.dma_start(out_d[st_i * 128:(st_i + 1) * 128, :],
                                      ysb[:, :])

    if split:
        _split_sync_waits(nc)
    return nc


def kernel(x, Wq, Wk, Wv, Wo, bo):
    x = np.asarray(x, dtype=np.float32)
    Wq = np.asarray(Wq, dtype=np.float32)
    Wk = np.asarray(Wk, dtype=np.float32)
    Wv = np.asarray(Wv, dtype=np.float32)
    Wo = np.asarray(Wo, dtype=np.float32)
    bo = np.asarray(bo, dtype=np.float32)

    if 'nc' not in _COMPILED:
        _COMPILED['nc'] = _build()
    nc = _COMPILED['nc']

    bf = ml_dtypes.bfloat16
    xT = np.ascontiguousarray(x.reshape(S, D).T).astype(bf)
    in_maps = []
    for h in range(N_CORES):
        sl = slice(h * DH, (h + 1) * DH)
        in_maps.append({
            'xT': xT,
            'wq': np.ascontiguousarray(Wq[sl, :].T).astype(bf),
            'wk': np.ascontiguousarray(Wk[sl, :].T).astype(bf),
            'wv': np.ascontiguousarray(Wv[sl, :].T).astype(bf),
            'woT': np.ascontiguousarray(Wo[:, sl].T).astype(bf),
        })

    trace = bool(os.environ.get('BASS_KERNEL_TRACE'))

    def _run():
        return bass_utils.run_bass_kernel_spmd(
            nc, in_maps, core_ids=list(range(N_CORES)), trace=trace,
            tmpdir=os.environ.get('BASS_KERNEL_TRACE_DIR') or None)

    try:
        res = _run()
    except Exception:
        # A previously crashed NEFF can leave the device unrecoverable; the
        # failed attempt clears it, so one retry is usually enough.
        res = _run()
    _COMPILED['last_res'] = res

    acc = res.results[0]['out'].astype(np.float32).copy()
    for h in range(1, N_CORES):
        acc += res.results[h]['out']
    acc += bo[None, :]
    return acc.reshape(1, S, D)


# revision 22
# speedup vs baseline: 1.0663x; 1.0261x over previous
"""CrossAttention (B=1, S=4096, H=8, DH=40) on 8 Trainium2 NeuronCores.

Sharding: tensor-parallel over the 8 heads — core h computes head h's full
attention plus its partial output projection; the host sums the 8 partials
and adds the bias.

Per-core dataflow (attention matmuls in bf16; fp32 accumulation in PSUM;
softmax renormalization cancels most of the bf16 rounding of P):
  qT/kT  [40, 4096] = Wq_h @ x.T            (PE, K=320 in 3 chunks)
  vT     [40, 4096] = Wv_h @ x.T            (PE)  -> PE-transposed to
  v'     [4096, 65]: cols 0..39 = v, col 64 = 1.0 (row-sum trick)
  ST     [128j, 512i] = k_j @ q_i.T         (PE, K=40)
  PT     = exp(ST / sqrt(40))               (ScalarE, PSUM->SBUF, bf16 out)
  O'.T|r [65, 512i] accumulate v'_j.T @ PT_j over j  (PE, K=128)
  rec    = 1/r (DVE), broadcast over partitions via K=1 matmul (fp32)
  oT     [40, 512i] = O'.T * rec            (DVE, bf16 out)
  Y_s    [128, 320] = oT_s.T @ Wo_h.T       (PE, K=40) -> DMA out
"""

import os

import ml_dtypes
import numpy as np

import concourse.bass as bass
import concourse.mybir as mybir
from concourse import bass_utils, masks
from concourse.tile import TileContext

S = 4096
D = 320
H = 8
DH = 40
N_CORES = 8
CHUNK = 512               # i-chunk width (one fp32 PSUM bank)
VW = 65                   # v' stationary width: 40 v cols, junk, ones col 64
GJ = 2                    # j-tiles per exp group (2 PSUM banks)
SCALE = float(DH) ** -0.5

F32 = mybir.dt.float32
F32R = mybir.dt.float32r
BF16 = mybir.dt.bfloat16
EXP = mybir.ActivationFunctionType.Exp

_COMPILED = {}


def _split_sync_waits(nc, max_waits=1):
    """This walrus build rejects instructions with more than one sync wait.
    Spill the excess onto same-engine nops placed just before the
    instruction (engine streams execute in program order, so all waits are
    satisfied before the instruction issues)."""
    for f in nc.m.functions:
        for bb in f.blocks:
            out = []
            changed = False
            for inst in bb.instructions:
                si = inst.sync_info
                if si is not None and si.on_wait and len(si.on_wait) > max_waits:
                    waits = list(si.on_wait)
                    for i in range(max_waits, len(waits), max_waits):
                        nop = mybir.InstNoOp(
                            name=nc.get_next_instruction_name(),
                            engine=inst.engine,
                            bass_nofuse=True,
                            sync_info=mybir.SyncInfo(
                                on_wait=waits[i:i + max_waits], on_update=[]),
                        )
                        out.append(nop)
                    inst.sync_info = mybir.SyncInfo(
                        on_wait=waits[:max_waits],
                        on_update=list(si.on_update or []))
                    changed = True
                out.append(inst)
            if changed:
                bb.instructions = out


def _build(s=None, split=True):
    s = s or S
    n_chunks = s // CHUNK
    jt = s // 128
    nc = bass.Bass('TRN2', target_bir_lowering=False, debug=False)

    xT_d = nc.dram_tensor('xT', [D, s], BF16, kind='ExternalInput').ap()
    wq_d = nc.dram_tensor('wq', [D, DH], BF16, kind='ExternalInput').ap()
    wk_d = nc.dram_tensor('wk', [D, DH], BF16, kind='ExternalInput').ap()
    wv_d = nc.dram_tensor('wv', [D, DH], BF16, kind='ExternalInput').ap()
    woT_d = nc.dram_tensor('woT', [DH, D], BF16, kind='ExternalInput').ap()
    out_d = nc.dram_tensor('out', [s, D], F32, kind='ExternalOutput').ap()

    KCH = (128, 128, 64)  # K chunks of D=320

    with TileContext(nc) as tc:
        with tc.tile_pool(name='const', bufs=1) as cpool, \
             tc.tile_pool(name='big', bufs=1) as big, \
             tc.tile_pool(name='pt', bufs=20) as ptp, \
             tc.tile_pool(name='work', bufs=3) as wkp, \
             tc.tile_pool(name='ps_st', bufs=1, space='PSUM') as ps_st, \
             tc.tile_pool(name='ps_small', bufs=2, space='PSUM') as ps_small, \
             tc.tile_pool(name='ps_av', bufs=1, space='PSUM') as ps_av:

            # ---- constants & inputs ----
            ident = cpool.tile([128, 128], F32, tag='ident')
            masks.make_identity(nc, ident[:, :])

            xt0 = big.tile([128, s], BF16, tag='xt0')
            xt1 = big.tile([128, s], BF16, tag='xt1')
            xt2 = big.tile([64, s], BF16, tag='xt2')
            xts = (xt0, xt1, xt2)
            # chunked loads so phase 1 starts before the whole xT lands
            for c in range(n_chunks):
                cs = slice(c * CHUNK, (c + 1) * CHUNK)
                nc.sync.dma_start(xt0[:, cs], xT_d[0:128, cs])
                nc.sync.dma_start(xt1[:, cs], xT_d[128:256, cs])
                nc.sync.dma_start(xt2[:, cs], xT_d[256:320, cs])

            # Combined projection stationaries for the 2x-packed QK^T:
            # wA = [wq | 0 | wk], wB = [wk | 0 | wq] (104 cols), giving
            # qkA = [q@0-39 | k@64-103] and qkB = [k@0-39 | q@64-103].
            QKW = 104
            wA = cpool.tile([128, 3 * QKW], BF16, tag='wA')
            wB = cpool.tile([128, 3 * QKW], BF16, tag='wB')
            nc.vector.memset(wA[:, :], 0.0)
            nc.vector.memset(wB[:, :], 0.0)
            wv_sb = cpool.tile([128, 3 * DH], BF16, tag='wv')
            for c, kk in enumerate(KCH):
                o = sum(KCH[:c])
                nc.sync.dma_start(wA[0:kk, c * QKW:c * QKW + DH],
                                  wq_d[o:o + kk, :])
                nc.sync.dma_start(wA[0:kk, c * QKW + 64:c * QKW + QKW],
                                  wk_d[o:o + kk, :])
                nc.sync.dma_start(wB[0:kk, c * QKW:c * QKW + DH],
                                  wk_d[o:o + kk, :])
                nc.sync.dma_start(wB[0:kk, c * QKW + 64:c * QKW + QKW],
                                  wq_d[o:o + kk, :])
                nc.sync.dma_start(wv_sb[0:kk, c * DH:(c + 1) * DH],
                                  wv_d[o:o + kk, :])
            woT_sb = cpool.tile([DH, D], BF16, tag='woT')
            nc.sync.dma_start(woT_sb[:, :], woT_d)

            qkA = big.tile([QKW, s], BF16, tag='qkA')
            qkB = big.tile([QKW, s], BF16, tag='qkB')
            # v plus zero rows 40..63 and ones row 64: transposing yields v'
            # tiles whose col 64 is 1.0 (the row-sum column).
            vT = big.tile([VW, s], F32, tag='vT')
            vsb = big.tile([128, jt * VW], BF16, tag='vsb')
            oT = big.tile([DH, s], BF16, tag='oT')

            # zero rows 32..63 first (32-aligned base); phase 1 then
            # overwrites rows 0..39 with v, leaving 40..63 zero
            nc.vector.memset(vT[32:VW - 1, :], 0.0)
            nc.vector.memset(vT[VW - 1:VW, :], 1.0)

            # ones at partition 64 for the rec broadcast matmul (fp32)
            ones64 = cpool.tile([65, DH], F32, tag='ones64')
            nc.vector.memset(ones64[64:65, :], 1.0)

            # ---- phase 1 helpers ----
            def proj(dst, w_sb, c, ww):
                ps = ps_small.tile([QKW, CHUNK], F32, tag='small')
                for ci, kk in enumerate(KCH):
                    nc.tensor.matmul(
                        ps[0:ww, :],
                        w_sb[0:kk, ci * ww:(ci + 1) * ww],
                        xts[ci][0:kk, c * CHUNK:(c + 1) * CHUNK],
                        start=(ci == 0), stop=(ci == 2))
                nc.vector.tensor_copy(dst[:, c * CHUNK:(c + 1) * CHUNK],
                                      ps[0:ww, :])

            def transpose_v(j):
                tp = ps_small.tile([128, VW], F32, tag='small')
                nc.tensor.transpose(tp[:, :], vT[:, j * 128:(j + 1) * 128],
                                    ident[0:VW, 0:VW])
                nc.vector.tensor_copy(vsb[:, j * VW:(j + 1) * VW], tp[:, :])

            # both combined tiles carry k, which every chunk's ST needs in
            # full — project both before the main loop
            for c in range(n_chunks):
                proj(qkB, wB, c, QKW)
            for c in range(n_chunks):
                proj(qkA, wA, c, QKW)

            # ---- main loop over i-chunks ----
            for c in range(n_chunks):
                pts = []
                cs = slice(c * CHUNK, (c + 1) * CHUNK)
                for g in range(jt // GJ):
                    st = ps_st.tile([128, GJ * CHUNK], F32, tag='st2')
                    j0, j1 = g * GJ, g * GJ + 1
                    # two K=40 matmuls on row groups 0 and 64 run concurrently
                    nc.tensor.matmul(
                        st[:, 0:CHUNK],
                        qkB[0:DH, j0 * 128:(j0 + 1) * 128], qkA[0:DH, cs],
                        start=True, stop=True)
                    nc.tensor.matmul(
                        st[:, CHUNK:2 * CHUNK],
                        qkA[64:QKW, j1 * 128:(j1 + 1) * 128], qkB[64:QKW, cs],
                        start=True, stop=True)
                    pt = ptp.tile([128, GJ * CHUNK], BF16, tag='pt')
                    nc.scalar.activation(pt[:, :], st[:, :], EXP, scale=SCALE)
                    pts.append(pt)

                if c == 0:
                    for cc in range(n_chunks):
                        proj(vT[0:DH, :], wv_sb, cc, DH)
                    for j in range(jt):
                        transpose_v(j)

                avq = [ps_av.tile([VW, CHUNK], F32, tag=f'av{q}',
                                  name=f'av{q}')
                       for q in range(2)]
                for j in range(jt):
                    ptt = pts[j // GJ]
                    pcs = slice((j % GJ) * CHUNK, (j % GJ + 1) * CHUNK)
                    for q in range(2):
                        nc.tensor.matmul(
                            avq[q][:, :],
                            vsb[64 * q:64 * (q + 1), j * VW:(j + 1) * VW],
                            ptt[64 * q:64 * (q + 1), pcs],
                            start=(j == 0), stop=(j == jt - 1),
                            tile_position=(64 * q, 0))

                # merge the 2 partial accumulators on DVE
                m1 = wkp.tile([VW, CHUNK], F32, tag='m1')
                nc.vector.tensor_copy(m1[:, :], avq[0][:, :])
                nc.vector.tensor_tensor(
                    out=m1[:, :], in0=avq[1][:, :], in1=m1[:, :],
                    op=mybir.AluOpType.add)

                rec = wkp.tile([65, CHUNK], F32, tag='rec')
                nc.vector.reciprocal(rec[64:65, :], m1[64:65, :])
                rbc = ps_small.tile([DH, CHUNK], F32, tag='small')
                nc.tensor.matmul(rbc[:, :], ones64[64:65, :],
                                 rec[64:65, :], start=True, stop=True)
                rbc_sb = wkp.tile([DH, CHUNK], F32, tag='rbc_sb')
                nc.vector.tensor_copy(rbc_sb[:, :], rbc[:, :])
                nc.vector.tensor_tensor(
                    out=oT[:, c * CHUNK:(c + 1) * CHUNK],
                    in0=m1[0:DH, :], in1=rbc_sb[:, :],
                    op=mybir.AluOpType.mult)

                # output projection for the 4 s-tiles of this chunk
                for s2 in range(CHUNK // 128):
                    st_i = c * (CHUNK // 128) + s2
                    yp = ps_small.tile([128, D], F32, tag='small')
                    nc.tensor.matmul(yp[:, :],
                                     oT[:, st_i * 128:(st_i + 1) * 128],
                                     woT_sb[:, :], start=True, stop=True)
                    ysb = wkp.tile([128, D], F32, tag='ysb')
                    nc.vector.tensor_copy(ysb[:, :], yp[:, :])
                    nc.sync.dma_start(out_d[st_i * 128:(st_i + 1) * 128, :],
                                      ysb[:, :])

    if split:
        _split_sync_waits(nc)
    return nc


def kernel(x, Wq, Wk, Wv, Wo, bo):
    x = np.asarray(x, dtype=np.float32)
    Wq = np.asarray(Wq, dtype=np.float32)
    Wk = np.asarray(Wk, dtype=np.float32)
    Wv = np.asarray(Wv, dtype=np.float32)
    Wo = np.asarray(Wo, dtype=np.float32)
    bo = np.asarray(bo, dtype=np.float32)

    if 'nc' not in _COMPILED:
        _COMPILED['nc'] = _build()
    nc = _COMPILED['nc']

    bf = ml_dtypes.bfloat16
    xT = np.ascontiguousarray(x.reshape(S, D).T).astype(bf)
    in_maps = []
    for h in range(N_CORES):
        sl = slice(h * DH, (h + 1) * DH)
        in_maps.append({
            'xT': xT,
            'wq': np.ascontiguousarray(Wq[sl, :].T).astype(bf),
            'wk': np.ascontiguousarray(Wk[sl, :].T).astype(bf),
            'wv': np.ascontiguousarray(Wv[sl, :].T).astype(bf),
            'woT': np.ascontiguousarray(Wo[:, sl].T).astype(bf),
        })

    trace = bool(os.environ.get('BASS_KERNEL_TRACE'))

    def _run():
        return bass_utils.run_bass_kernel_spmd(
            nc, in_maps, core_ids=list(range(N_CORES)), trace=trace,
            tmpdir=os.environ.get('BASS_KERNEL_TRACE_DIR') or None)

    try:
        res = _run()
    except Exception:
        # A previously crashed NEFF can leave the device unrecoverable; the
        # failed attempt clears it, so one retry is usually enough.
        res = _run()
    _COMPILED['last_res'] = res

    acc = res.results[0]['out'].astype(np.float32).copy()
    for h in range(1, N_CORES):
        acc += res.results[h]['out']
    acc += bo[None, :]
    return acc.reshape(1, S, D)


# revision 23
# speedup vs baseline: 1.3129x; 1.2312x over previous
"""CrossAttention (B=1, S=4096, H=8, DH=40) on 8 Trainium2 NeuronCores.

Sharding: tensor-parallel over the 8 heads — core h computes head h's full
attention plus its partial output projection; the host sums the 8 partials
and adds the bias.

Per-core dataflow (attention matmuls in bf16; fp32 accumulation in PSUM;
softmax renormalization cancels most of the bf16 rounding of P):
  qT/kT  [40, 4096] = Wq_h @ x.T            (PE, K=320 in 3 chunks)
  vT     [40, 4096] = Wv_h @ x.T            (PE)  -> PE-transposed to
  v'     [4096, 65]: cols 0..39 = v, col 64 = 1.0 (row-sum trick)
  ST     [128j, 512i] = k_j @ q_i.T         (PE, K=40)
  PT     = exp(ST / sqrt(40))               (ScalarE, PSUM->SBUF, bf16 out)
  O'.T|r [65, 512i] accumulate v'_j.T @ PT_j over j  (PE, K=128)
  rec    = 1/r (DVE), broadcast over partitions via K=1 matmul (fp32)
  oT     [40, 512i] = O'.T * rec            (DVE, bf16 out)
  Y_s    [128, 320] = oT_s.T @ Wo_h.T       (PE, K=40) -> DMA out
"""

import os

import ml_dtypes
import numpy as np

import concourse.bass as bass
import concourse.mybir as mybir
from concourse import bass_utils, masks
from concourse.tile import TileContext

S = 4096
D = 320
H = 8
DH = 40
N_CORES = 8
CHUNK = 512               # i-chunk width (one fp32 PSUM bank)
VW = 65                   # v' stationary width: 40 v cols, junk, ones col 64
GJ = 2                    # j-tiles per exp group (2 PSUM banks)
SCALE = float(DH) ** -0.5

F32 = mybir.dt.float32
F32R = mybir.dt.float32r
BF16 = mybir.dt.bfloat16
EXP = mybir.ActivationFunctionType.Exp

_COMPILED = {}


def _split_sync_waits(nc, max_waits=1):
    """This walrus build rejects instructions with more than one sync wait.
    Spill the excess onto same-engine nops placed just before the
    instruction (engine streams execute in program order, so all waits are
    satisfied before the instruction issues)."""
    for f in nc.m.functions:
        for bb in f.blocks:
            out = []
            changed = False
            for inst in bb.instructions:
                si = inst.sync_info
                if si is not None and si.on_wait and len(si.on_wait) > max_waits:
                    waits = list(si.on_wait)
                    for i in range(max_waits, len(waits), max_waits):
                        nop = mybir.InstNoOp(
                            name=nc.get_next_instruction_name(),
                            engine=inst.engine,
                            bass_nofuse=True,
                            sync_info=mybir.SyncInfo(
                                on_wait=waits[i:i + max_waits], on_update=[]),
                        )
                        out.append(nop)
                    inst.sync_info = mybir.SyncInfo(
                        on_wait=waits[:max_waits],
                        on_update=list(si.on_update or []))
                    changed = True
                out.append(inst)
            if changed:
                bb.instructions = out


def _build(s=None, split=True):
    s = s or S
    n_chunks = s // CHUNK
    jt = s // 128
    nc = bass.Bass('TRN2', target_bir_lowering=False, debug=False)

    xT_d = nc.dram_tensor('xT', [D, s], BF16, kind='ExternalInput').ap()
    wq_d = nc.dram_tensor('wq', [D, DH], BF16, kind='ExternalInput').ap()
    wk_d = nc.dram_tensor('wk', [D, DH], BF16, kind='ExternalInput').ap()
    wv_d = nc.dram_tensor('wv', [D, DH], BF16, kind='ExternalInput').ap()
    woT_d = nc.dram_tensor('woT', [DH, D], BF16, kind='ExternalInput').ap()
    out_d = nc.dram_tensor('out', [s, D], F32, kind='ExternalOutput').ap()

    KCH = (128, 128, 64)  # K chunks of D=320

    with TileContext(nc) as tc:
        with tc.tile_pool(name='const', bufs=1) as cpool, \
             tc.tile_pool(name='big', bufs=1) as big, \
             tc.tile_pool(name='pt', bufs=20) as ptp, \
             tc.tile_pool(name='work', bufs=3) as wkp, \
             tc.tile_pool(name='ps_st', bufs=2, space='PSUM') as ps_st, \
             tc.tile_pool(name='ps_small', bufs=2, space='PSUM') as ps_small, \
             tc.tile_pool(name='ps_av', bufs=1, space='PSUM') as ps_av:

            # ---- constants & inputs ----
            ident = cpool.tile([128, 128], F32, tag='ident')
            masks.make_identity(nc, ident[:, :])

            xt0 = big.tile([128, s], BF16, tag='xt0')
            xt1 = big.tile([128, s], BF16, tag='xt1')
            xt2 = big.tile([64, s], BF16, tag='xt2')
            xts = (xt0, xt1, xt2)
            # chunked loads so phase 1 starts before the whole xT lands
            for c in range(n_chunks):
                cs = slice(c * CHUNK, (c + 1) * CHUNK)
                nc.sync.dma_start(xt0[:, cs], xT_d[0:128, cs])
                nc.sync.dma_start(xt1[:, cs], xT_d[128:256, cs])
                nc.sync.dma_start(xt2[:, cs], xT_d[256:320, cs])

            # Combined projection stationaries for the 2x-packed QK^T:
            # wA = [wq | 0 | wk], wB = [wk | 0 | wq] (104 cols), giving
            # qkA = [q@0-39 | k@64-103] and qkB = [k@0-39 | q@64-103].
            QKW = 104
            wA = cpool.tile([128, 3 * QKW], BF16, tag='wA')
            wB = cpool.tile([128, 3 * QKW], BF16, tag='wB')
            nc.vector.memset(wA[:, :], 0.0)
            nc.vector.memset(wB[:, :], 0.0)
            wv_sb = cpool.tile([128, 3 * DH], BF16, tag='wv')
            for c, kk in enumerate(KCH):
                o = sum(KCH[:c])
                nc.sync.dma_start(wA[0:kk, c * QKW:c * QKW + DH],
                                  wq_d[o:o + kk, :])
                nc.sync.dma_start(wA[0:kk, c * QKW + 64:c * QKW + QKW],
                                  wk_d[o:o + kk, :])
                nc.sync.dma_start(wB[0:kk, c * QKW:c * QKW + DH],
                                  wk_d[o:o + kk, :])
                nc.sync.dma_start(wB[0:kk, c * QKW + 64:c * QKW + QKW],
                                  wq_d[o:o + kk, :])
                nc.sync.dma_start(wv_sb[0:kk, c * DH:(c + 1) * DH],
                                  wv_d[o:o + kk, :])
            woT_sb = cpool.tile([DH, D], BF16, tag='woT')
            nc.sync.dma_start(woT_sb[:, :], woT_d)

            qkA = big.tile([QKW, s], BF16, tag='qkA')
            qkB = big.tile([QKW, s], BF16, tag='qkB')
            # v plus zero rows 40..63 and ones row 64: transposing yields v'
            # tiles whose col 64 is 1.0 (the row-sum column).
            vT = big.tile([VW, s], F32, tag='vT')
            vsb = big.tile([128, jt * VW], BF16, tag='vsb')
            oT = big.tile([DH, s], BF16, tag='oT')

            # zero rows 32..63 first (32-aligned base); phase 1 then
            # overwrites rows 0..39 with v, leaving 40..63 zero
            nc.vector.memset(vT[32:VW - 1, :], 0.0)
            nc.vector.memset(vT[VW - 1:VW, :], 1.0)

            # ones at partition 64 for the rec broadcast matmul (fp32)
            ones64 = cpool.tile([65, DH], F32, tag='ones64')
            nc.vector.memset(ones64[64:65, :], 1.0)

            # ---- phase 1 helpers ----
            def proj(dst, w_sb, c, ww):
                ps = ps_small.tile([QKW, CHUNK], F32, tag='small')
                for ci, kk in enumerate(KCH):
                    nc.tensor.matmul(
                        ps[0:ww, :],
                        w_sb[0:kk, ci * ww:(ci + 1) * ww],
                        xts[ci][0:kk, c * CHUNK:(c + 1) * CHUNK],
                        start=(ci == 0), stop=(ci == 2))
                nc.vector.tensor_copy(dst[:, c * CHUNK:(c + 1) * CHUNK],
                                      ps[0:ww, :])

            def transpose_v(j):
                tp = ps_small.tile([128, VW], F32, tag='small')
                nc.tensor.transpose(tp[:, :], vT[:, j * 128:(j + 1) * 128],
                                    ident[0:VW, 0:VW])
                nc.vector.tensor_copy(vsb[:, j * VW:(j + 1) * VW], tp[:, :])

            # qkB (k@0) fully first plus qkA(0): chunk 0 runs its ST
            # unpacked from base-0 operands so exp starts early, while the
            # remaining qkA chunks (k@64 for the packed pairs) project in
            # the background during chunk 0
            for c in range(n_chunks):
                proj(qkB, wB, c, QKW)
            proj(qkA, wA, 0, QKW)

            # ---- main loop over i-chunks ----
            for c in range(n_chunks):
                pts = []
                cs = slice(c * CHUNK, (c + 1) * CHUNK)
                for g in range(jt // GJ):
                    st = ps_st.tile([128, GJ * CHUNK], F32, tag='st2')
                    j0, j1 = g * GJ, g * GJ + 1
                    # chunk 0: both matmuls from base-0 operands (qkA k@64
                    # not projected yet); later chunks: row groups 0 and 64
                    # run concurrently
                    nc.tensor.matmul(
                        st[:, 0:CHUNK],
                        qkB[0:DH, j0 * 128:(j0 + 1) * 128], qkA[0:DH, cs],
                        start=True, stop=True)
                    if c == 0:
                        nc.tensor.matmul(
                            st[:, CHUNK:2 * CHUNK],
                            qkB[0:DH, j1 * 128:(j1 + 1) * 128], qkA[0:DH, cs],
                            start=True, stop=True)
                    else:
                        nc.tensor.matmul(
                            st[:, CHUNK:2 * CHUNK],
                            qkA[64:QKW, j1 * 128:(j1 + 1) * 128],
                            qkB[64:QKW, cs],
                            start=True, stop=True)
                    pt = ptp.tile([128, GJ * CHUNK], BF16, tag='pt')
                    nc.scalar.activation(pt[:, :], st[:, :], EXP, scale=SCALE)
                    pts.append(pt)

                if c == 0:
                    for cc in range(1, n_chunks):
                        proj(qkA, wA, cc, QKW)
                    for cc in range(n_chunks):
                        proj(vT[0:DH, :], wv_sb, cc, DH)
                    for j in range(jt):
                        transpose_v(j)

                avq = [ps_av.tile([VW, CHUNK], F32, tag=f'av{q}',
                                  name=f'av{q}')
                       for q in range(2)]
                for j in range(jt):
                    ptt = pts[j // GJ]
                    pcs = slice((j % GJ) * CHUNK, (j % GJ + 1) * CHUNK)
                    for q in range(2):
                        nc.tensor.matmul(
                            avq[q][:, :],
                            vsb[64 * q:64 * (q + 1), j * VW:(j + 1) * VW],
                            ptt[64 * q:64 * (q + 1), pcs],
                            start=(j == 0), stop=(j == jt - 1),
                            tile_position=(64 * q, 0))

                # merge the 2 partial accumulators on DVE
                m1 = wkp.tile([VW, CHUNK], F32, tag='m1')
                nc.vector.tensor_copy(m1[:, :], avq[0][:, :])
                nc.vector.tensor_tensor(
                    out=m1[:, :], in0=avq[1][:, :], in1=m1[:, :],
                    op=mybir.AluOpType.add)

                rec = wkp.tile([65, CHUNK], F32, tag='rec')
                nc.vector.reciprocal(rec[64:65, :], m1[64:65, :])
                rbc = ps_small.tile([DH, CHUNK], F32, tag='small')
                nc.tensor.matmul(rbc[:, :], ones64[64:65, :],
                                 rec[64:65, :], start=True, stop=True)
                rbc_sb = wkp.tile([DH, CHUNK], F32, tag='rbc_sb')
                nc.vector.tensor_copy(rbc_sb[:, :], rbc[:, :])
                nc.vector.tensor_tensor(
                    out=oT[:, c * CHUNK:(c + 1) * CHUNK],
                    in0=m1[0:DH, :], in1=rbc_sb[:, :],
                    op=mybir.AluOpType.mult)

                # output projection for the 4 s-tiles of this chunk
                for s2 in range(CHUNK // 128):
                    st_i = c * (CHUNK // 128) + s2
                    yp = ps_small.tile([128, D], F32, tag='small')
                    nc.tensor.matmul(yp[:, :],
                                     oT[:, st_i * 128:(st_i + 1) * 128],
                                     woT_sb[:, :], start=True, stop=True)
                    ysb = wkp.tile([128, D], F32, tag='ysb')
                    nc.vector.tensor_copy(ysb[:, :], yp[:, :])
                    nc.sync.dma_start(out_d[st_i * 128:(st_i + 1) * 128, :],
                                      ysb[:, :])

    if split:
        _split_sync_waits(nc)
    return nc


def kernel(x, Wq, Wk, Wv, Wo, bo):
    x = np.asarray(x, dtype=np.float32)
    Wq = np.asarray(Wq, dtype=np.float32)
    Wk = np.asarray(Wk, dtype=np.float32)
    Wv = np.asarray(Wv, dtype=np.float32)
    Wo = np.asarray(Wo, dtype=np.float32)
    bo = np.asarray(bo, dtype=np.float32)

    if 'nc' not in _COMPILED:
        _COMPILED['nc'] = _build()
    nc = _COMPILED['nc']

    bf = ml_dtypes.bfloat16
    xT = np.ascontiguousarray(x.reshape(S, D).T).astype(bf)
    in_maps = []
    for h in range(N_CORES):
        sl = slice(h * DH, (h + 1) * DH)
        in_maps.append({
            'xT': xT,
            'wq': np.ascontiguousarray(Wq[sl, :].T).astype(bf),
            'wk': np.ascontiguousarray(Wk[sl, :].T).astype(bf),
            'wv': np.ascontiguousarray(Wv[sl, :].T).astype(bf),
            'woT': np.ascontiguousarray(Wo[:, sl].T).astype(bf),
        })

    trace = bool(os.environ.get('BASS_KERNEL_TRACE'))

    def _run():
        return bass_utils.run_bass_kernel_spmd(
            nc, in_maps, core_ids=list(range(N_CORES)), trace=trace,
            tmpdir=os.environ.get('BASS_KERNEL_TRACE_DIR') or None)

    try:
        res = _run()
    except Exception:
        # A previously crashed NEFF can leave the device unrecoverable; the
        # failed attempt clears it, so one retry is usually enough.
        res = _run()
    _COMPILED['last_res'] = res

    acc = res.results[0]['out'].astype(np.float32).copy()
    for h in range(1, N_CORES):
        acc += res.results[h]['out']
    acc += bo[None, :]
    return acc.reshape(1, S, D)


# revision 24
# speedup vs baseline: 1.3500x; 1.0283x over previous
"""CrossAttention (B=1, S=4096, H=8, DH=40) on 8 Trainium2 NeuronCores.

Sharding: tensor-parallel over the 8 heads — core h computes head h's full
attention plus its partial output projection; the host sums the 8 partials
and adds the bias.

Per-core dataflow (attention matmuls in bf16; fp32 accumulation in PSUM;
softmax renormalization cancels most of the bf16 rounding of P):
  qT/kT  [40, 4096] = Wq_h @ x.T            (PE, K=320 in 3 chunks)
  vT     [40, 4096] = Wv_h @ x.T            (PE)  -> PE-transposed to
  v'     [4096, 65]: cols 0..39 = v, col 64 = 1.0 (row-sum trick)
  ST     [128j, 512i] = k_j @ q_i.T         (PE, K=40)
  PT     = exp(ST / sqrt(40))               (ScalarE, PSUM->SBUF, bf16 out)
  O'.T|r [65, 512i] accumulate v'_j.T @ PT_j over j  (PE, K=128)
  rec    = 1/r (DVE), broadcast over partitions via K=1 matmul (fp32)
  oT     [40, 512i] = O'.T * rec            (DVE, bf16 out)
  Y_s    [128, 320] = oT_s.T @ Wo_h.T       (PE, K=40) -> DMA out
"""

import os

import ml_dtypes
import numpy as np

import concourse.bass as bass
import concourse.mybir as mybir
from concourse import bass_utils, masks
from concourse.tile import TileContext

S = 4096
D = 320
H = 8
DH = 40
N_CORES = 8
CHUNK = 512               # i-chunk width (one fp32 PSUM bank)
VW = 65                   # v' stationary width: 40 v cols, junk, ones col 64
GJ = 2                    # j-tiles per exp group (2 PSUM banks)
SCALE = float(DH) ** -0.5

F32 = mybir.dt.float32
F32R = mybir.dt.float32r
BF16 = mybir.dt.bfloat16
EXP = mybir.ActivationFunctionType.Exp

_COMPILED = {}


def _split_sync_waits(nc, max_waits=1):
    """This walrus build rejects instructions with more than one sync wait.
    Spill the excess onto same-engine nops placed just before the
    instruction (engine streams execute in program order, so all waits are
    satisfied before the instruction issues)."""
    for f in nc.m.functions:
        for bb in f.blocks:
            out = []
            changed = False
            for inst in bb.instructions:
                si = inst.sync_info
                if si is not None and si.on_wait and len(si.on_wait) > max_waits:
                    waits = list(si.on_wait)
                    for i in range(max_waits, len(waits), max_waits):
                        nop = mybir.InstNoOp(
                            name=nc.get_next_instruction_name(),
                            engine=inst.engine,
                            bass_nofuse=True,
                            sync_info=mybir.SyncInfo(
                                on_wait=waits[i:i + max_waits], on_update=[]),
                        )
                        out.append(nop)
                    inst.sync_info = mybir.SyncInfo(
                        on_wait=waits[:max_waits],
                        on_update=list(si.on_update or []))
                    changed = True
                out.append(inst)
            if changed:
                bb.instructions = out


def _build(s=None, split=True):
    s = s or S
    n_chunks = s // CHUNK
    jt = s // 128
    nc = bass.Bass('TRN2', target_bir_lowering=False, debug=False)

    xT_d = nc.dram_tensor('xT', [D, s], BF16, kind='ExternalInput').ap()
    wq_d = nc.dram_tensor('wq', [D, DH], BF16, kind='ExternalInput').ap()
    wk_d = nc.dram_tensor('wk', [D, DH], BF16, kind='ExternalInput').ap()
    wv_d = nc.dram_tensor('wv', [D, DH], BF16, kind='ExternalInput').ap()
    woT_d = nc.dram_tensor('woT', [DH, D], BF16, kind='ExternalInput').ap()
    out_d = nc.dram_tensor('out', [s, D], F32, kind='ExternalOutput').ap()

    KCH = (128, 128, 64)  # K chunks of D=320

    with TileContext(nc) as tc:
        with tc.tile_pool(name='const', bufs=1) as cpool, \
             tc.tile_pool(name='big', bufs=1) as big, \
             tc.tile_pool(name='pt', bufs=20) as ptp, \
             tc.tile_pool(name='work', bufs=3) as wkp, \
             tc.tile_pool(name='ps_st', bufs=2, space='PSUM') as ps_st, \
             tc.tile_pool(name='ps_small', bufs=2, space='PSUM') as ps_small, \
             tc.tile_pool(name='ps_av', bufs=1, space='PSUM') as ps_av:

            # ---- constants & inputs ----
            ident = cpool.tile([128, 128], F32, tag='ident')
            masks.make_identity(nc, ident[:, :])

            # Combined projection stationaries for the 2x-packed QK^T:
            # wA = [wq | 0 | wk], wB = [wk | 0 | wq] (104 cols), giving
            # qkA = [q@0-39 | k@64-103] and qkB = [k@0-39 | q@64-103].
            QKW = 104
            wA = cpool.tile([128, 3 * QKW], BF16, tag='wA')
            wB = cpool.tile([128, 3 * QKW], BF16, tag='wB')
            nc.vector.memset(wA[:, :], 0.0)
            nc.vector.memset(wB[:, :], 0.0)
            wv_sb = cpool.tile([128, 3 * DH], BF16, tag='wv')
            for c, kk in enumerate(KCH):
                o = sum(KCH[:c])
                nc.sync.dma_start(wA[0:kk, c * QKW:c * QKW + DH],
                                  wq_d[o:o + kk, :])
                nc.sync.dma_start(wA[0:kk, c * QKW + 64:c * QKW + QKW],
                                  wk_d[o:o + kk, :])
                nc.sync.dma_start(wB[0:kk, c * QKW:c * QKW + DH],
                                  wk_d[o:o + kk, :])
                nc.sync.dma_start(wB[0:kk, c * QKW + 64:c * QKW + QKW],
                                  wq_d[o:o + kk, :])
                nc.sync.dma_start(wv_sb[0:kk, c * DH:(c + 1) * DH],
                                  wv_d[o:o + kk, :])
            woT_sb = cpool.tile([DH, D], BF16, tag='woT')
            nc.sync.dma_start(woT_sb[:, :], woT_d)

            xt0 = big.tile([128, s], BF16, tag='xt0')
            xt1 = big.tile([128, s], BF16, tag='xt1')
            xt2 = big.tile([64, s], BF16, tag='xt2')
            xts = (xt0, xt1, xt2)
            # chunked loads so phase 1 starts before the whole xT lands
            for c in range(n_chunks):
                cs = slice(c * CHUNK, (c + 1) * CHUNK)
                nc.sync.dma_start(xt0[:, cs], xT_d[0:128, cs])
                nc.sync.dma_start(xt1[:, cs], xT_d[128:256, cs])
                nc.sync.dma_start(xt2[:, cs], xT_d[256:320, cs])


            qkA = big.tile([QKW, s], BF16, tag='qkA')
            qkB = big.tile([QKW, s], BF16, tag='qkB')
            # v plus zero rows 40..63 and ones row 64: transposing yields v'
            # tiles whose col 64 is 1.0 (the row-sum column).
            vT = big.tile([VW, s], F32, tag='vT')
            vsb = big.tile([128, jt * VW], BF16, tag='vsb')
            oT = big.tile([DH, s], BF16, tag='oT')

            # zero rows 32..63 first (32-aligned base); phase 1 then
            # overwrites rows 0..39 with v, leaving 40..63 zero
            nc.vector.memset(vT[32:VW - 1, :], 0.0)
            nc.vector.memset(vT[VW - 1:VW, :], 1.0)

            # ones at partition 64 for the rec broadcast matmul (fp32)
            ones64 = cpool.tile([65, DH], F32, tag='ones64')
            nc.vector.memset(ones64[64:65, :], 1.0)

            # ---- phase 1 helpers ----
            def proj(dst, w_sb, c, ww):
                ps = ps_small.tile([QKW, CHUNK], F32, tag='small')
                for ci, kk in enumerate(KCH):
                    nc.tensor.matmul(
                        ps[0:ww, :],
                        w_sb[0:kk, ci * ww:(ci + 1) * ww],
                        xts[ci][0:kk, c * CHUNK:(c + 1) * CHUNK],
                        start=(ci == 0), stop=(ci == 2))
                nc.vector.tensor_copy(dst[:, c * CHUNK:(c + 1) * CHUNK],
                                      ps[0:ww, :])

            def transpose_v(j):
                tp = ps_small.tile([128, VW], F32, tag='small')
                nc.tensor.transpose(tp[:, :], vT[:, j * 128:(j + 1) * 128],
                                    ident[0:VW, 0:VW])
                nc.vector.tensor_copy(vsb[:, j * VW:(j + 1) * VW], tp[:, :])

            # qkB (k@0) fully first plus qkA(0): chunk 0 runs its ST
            # unpacked from base-0 operands so exp starts early, while the
            # remaining qkA chunks (k@64 for the packed pairs) project in
            # the background during chunk 0
            for c in range(n_chunks):
                proj(qkB, wB, c, QKW)
            proj(qkA, wA, 0, QKW)

            # ---- main loop over i-chunks ----
            for c in range(n_chunks):
                pts = []
                cs = slice(c * CHUNK, (c + 1) * CHUNK)
                for g in range(jt // GJ):
                    st = ps_st.tile([128, GJ * CHUNK], F32, tag='st2')
                    j0, j1 = g * GJ, g * GJ + 1
                    # chunk 0: both matmuls from base-0 operands (qkA k@64
                    # not projected yet); later chunks: row groups 0 and 64
                    # run concurrently
                    nc.tensor.matmul(
                        st[:, 0:CHUNK],
                        qkB[0:DH, j0 * 128:(j0 + 1) * 128], qkA[0:DH, cs],
                        start=True, stop=True)
                    if c == 0:
                        nc.tensor.matmul(
                            st[:, CHUNK:2 * CHUNK],
                            qkB[0:DH, j1 * 128:(j1 + 1) * 128], qkA[0:DH, cs],
                            start=True, stop=True)
                    else:
                        nc.tensor.matmul(
                            st[:, CHUNK:2 * CHUNK],
                            qkA[64:QKW, j1 * 128:(j1 + 1) * 128],
                            qkB[64:QKW, cs],
                            start=True, stop=True)
                    pt = ptp.tile([128, GJ * CHUNK], BF16, tag='pt')
                    nc.scalar.activation(pt[:, :], st[:, :], EXP, scale=SCALE)
                    pts.append(pt)

                if c == 0:
                    for cc in range(1, n_chunks):
                        proj(qkA, wA, cc, QKW)
                    for cc in range(n_chunks):
                        proj(vT[0:DH, :], wv_sb, cc, DH)
                    for j in range(jt):
                        transpose_v(j)

                avq = [ps_av.tile([VW, CHUNK], F32, tag=f'av{q}',
                                  name=f'av{q}')
                       for q in range(2)]
                for j in range(jt):
                    ptt = pts[j // GJ]
                    pcs = slice((j % GJ) * CHUNK, (j % GJ + 1) * CHUNK)
                    for q in range(2):
                        nc.tensor.matmul(
                            avq[q][:, :],
                            vsb[64 * q:64 * (q + 1), j * VW:(j + 1) * VW],
                            ptt[64 * q:64 * (q + 1), pcs],
                            start=(j == 0), stop=(j == jt - 1),
                            tile_position=(64 * q, 0))

                # merge the 2 partial accumulators on DVE
                m1 = wkp.tile([VW, CHUNK], F32, tag='m1')
                nc.vector.tensor_copy(m1[:, :], avq[0][:, :])
                nc.vector.tensor_tensor(
                    out=m1[:, :], in0=avq[1][:, :], in1=m1[:, :],
                    op=mybir.AluOpType.add)

                rec = wkp.tile([65, CHUNK], F32, tag='rec')
                nc.vector.reciprocal(rec[64:65, :], m1[64:65, :])
                rbc = ps_small.tile([DH, CHUNK], F32, tag='small')
                nc.tensor.matmul(rbc[:, :], ones64[64:65, :],
                                 rec[64:65, :], start=True, stop=True)
                rbc_sb = wkp.tile([DH, CHUNK], F32, tag='rbc_sb')
                nc.vector.tensor_copy(rbc_sb[:, :], rbc[:, :])
                nc.vector.tensor_tensor(
                    out=oT[:, c * CHUNK:(c + 1) * CHUNK],
                    in0=m1[0:DH, :], in1=rbc_sb[:, :],
                    op=mybir.AluOpType.mult)

                # output projection for the 4 s-tiles of this chunk
                for s2 in range(CHUNK // 128):
                    st_i = c * (CHUNK // 128) + s2
                    yp = ps_small.tile([128, D], F32, tag='small')
                    nc.tensor.matmul(yp[:, :],
                                     oT[:, st_i * 128:(st_i + 1) * 128],
                                     woT_sb[:, :], start=True, stop=True)
                    ysb = wkp.tile([128, D], F32, tag='ysb')
                    nc.vector.tensor_copy(ysb[:, :], yp[:, :])
                    nc.sync.dma_start(out_d[st_i * 128:(st_i + 1) * 128, :],
                                      ysb[:, :])

    if split:
        _split_sync_waits(nc)
    return nc


def kernel(x, Wq, Wk, Wv, Wo, bo):
    x = np.asarray(x, dtype=np.float32)
    Wq = np.asarray(Wq, dtype=np.float32)
    Wk = np.asarray(Wk, dtype=np.float32)
    Wv = np.asarray(Wv, dtype=np.float32)
    Wo = np.asarray(Wo, dtype=np.float32)
    bo = np.asarray(bo, dtype=np.float32)

    if 'nc' not in _COMPILED:
        _COMPILED['nc'] = _build()
    nc = _COMPILED['nc']

    bf = ml_dtypes.bfloat16
    xT = np.ascontiguousarray(x.reshape(S, D).T).astype(bf)
    in_maps = []
    for h in range(N_CORES):
        sl = slice(h * DH, (h + 1) * DH)
        in_maps.append({
            'xT': xT,
            'wq': np.ascontiguousarray(Wq[sl, :].T).astype(bf),
            'wk': np.ascontiguousarray(Wk[sl, :].T).astype(bf),
            'wv': np.ascontiguousarray(Wv[sl, :].T).astype(bf),
            'woT': np.ascontiguousarray(Wo[:, sl].T).astype(bf),
        })

    trace = bool(os.environ.get('BASS_KERNEL_TRACE'))

    def _run():
        return bass_utils.run_bass_kernel_spmd(
            nc, in_maps, core_ids=list(range(N_CORES)), trace=trace,
            tmpdir=os.environ.get('BASS_KERNEL_TRACE_DIR') or None)

    try:
        res = _run()
    except Exception:
        # A previously crashed NEFF can leave the device unrecoverable; the
        # failed attempt clears it, so one retry is usually enough.
        res = _run()
    _COMPILED['last_res'] = res

    acc = res.results[0]['out'].astype(np.float32).copy()
    for h in range(1, N_CORES):
        acc += res.results[h]['out']
    acc += bo[None, :]
    return acc.reshape(1, S, D)


# revision 25
# speedup vs baseline: 1.4441x; 1.0697x over previous
"""CrossAttention (B=1, S=4096, H=8, DH=40) on 8 Trainium2 NeuronCores.

Sharding: tensor-parallel over the 8 heads — core h computes head h's full
attention plus its partial output projection; the host sums the 8 partials
and adds the bias.

Per-core dataflow (attention matmuls in bf16; fp32 accumulation in PSUM;
softmax renormalization cancels most of the bf16 rounding of P):
  qT/kT  [40, 4096] = Wq_h @ x.T            (PE, K=320 in 3 chunks)
  vT     [40, 4096] = Wv_h @ x.T            (PE)  -> PE-transposed to
  v'     [4096, 65]: cols 0..39 = v, col 64 = 1.0 (row-sum trick)
  ST     [128j, 512i] = k_j @ q_i.T         (PE, K=40)
  PT     = exp(ST / sqrt(40))               (ScalarE, PSUM->SBUF, bf16 out)
  O'.T|r [65, 512i] accumulate v'_j.T @ PT_j over j  (PE, K=128)
  rec    = 1/r (DVE), broadcast over partitions via K=1 matmul (fp32)
  oT     [40, 512i] = O'.T * rec            (DVE, bf16 out)
  Y_s    [128, 320] = oT_s.T @ Wo_h.T       (PE, K=40) -> DMA out
"""

import os

import ml_dtypes
import numpy as np

import concourse.bass as bass
import concourse.mybir as mybir
from concourse import bass_utils, masks
from concourse.tile import TileContext

S = 4096
D = 320
H = 8
DH = 40
N_CORES = 8
CHUNK = 512               # i-chunk width (one fp32 PSUM bank)
VW = 65                   # v' stationary width: 40 v cols, junk, ones col 64
GJ = 2                    # j-tiles per exp group (2 PSUM banks)
SCALE = float(DH) ** -0.5

F32 = mybir.dt.float32
F32R = mybir.dt.float32r
BF16 = mybir.dt.bfloat16
EXP = mybir.ActivationFunctionType.Exp

_COMPILED = {}


def _split_sync_waits(nc, max_waits=1):
    """This walrus build rejects instructions with more than one sync wait.
    Spill the excess onto same-engine nops placed just before the
    instruction (engine streams execute in program order, so all waits are
    satisfied before the instruction issues)."""
    for f in nc.m.functions:
        for bb in f.blocks:
            out = []
            changed = False
            for inst in bb.instructions:
                si = inst.sync_info
                if si is not None and si.on_wait and len(si.on_wait) > max_waits:
                    waits = list(si.on_wait)
                    for i in range(max_waits, len(waits), max_waits):
                        nop = mybir.InstNoOp(
                            name=nc.get_next_instruction_name(),
                            engine=inst.engine,
                            bass_nofuse=True,
                            sync_info=mybir.SyncInfo(
                                on_wait=waits[i:i + max_waits], on_update=[]),
                        )
                        out.append(nop)
                    inst.sync_info = mybir.SyncInfo(
                        on_wait=waits[:max_waits],
                        on_update=list(si.on_update or []))
                    changed = True
                out.append(inst)
            if changed:
                bb.instructions = out


def _build(s=None, split=True):
    s = s or S
    n_chunks = s // CHUNK
    jt = s // 128
    nc = bass.Bass('TRN2', target_bir_lowering=False, debug=False)

    xT_d = nc.dram_tensor('xT', [D, s], BF16, kind='ExternalInput').ap()
    wq_d = nc.dram_tensor('wq', [D, DH], BF16, kind='ExternalInput').ap()
    wk_d = nc.dram_tensor('wk', [D, DH], BF16, kind='ExternalInput').ap()
    wv_d = nc.dram_tensor('wv', [D, DH], BF16, kind='ExternalInput').ap()
    woT_d = nc.dram_tensor('woT', [DH, D], BF16, kind='ExternalInput').ap()
    out_d = nc.dram_tensor('out', [s, D], F32, kind='ExternalOutput').ap()

    KCH = (128, 128, 64)  # K chunks of D=320

    with TileContext(nc) as tc:
        with tc.tile_pool(name='const', bufs=1) as cpool, \
             tc.tile_pool(name='big', bufs=1) as big, \
             tc.tile_pool(name='pt', bufs=36) as ptp, \
             tc.tile_pool(name='work', bufs=3) as wkp, \
             tc.tile_pool(name='ps_st', bufs=2, space='PSUM') as ps_st, \
             tc.tile_pool(name='ps_small', bufs=2, space='PSUM') as ps_small, \
             tc.tile_pool(name='ps_av', bufs=1, space='PSUM') as ps_av:

            # ---- constants & inputs ----
            ident = cpool.tile([128, 128], F32, tag='ident')
            masks.make_identity(nc, ident[:, :])

            # Combined projection stationaries for the 2x-packed QK^T:
            # wA = [wq | 0 | wk], wB = [wk | 0 | wq] (104 cols), giving
            # qkA = [q@0-39 | k@64-103] and qkB = [k@0-39 | q@64-103].
            QKW = 104
            wA = cpool.tile([128, 3 * QKW], BF16, tag='wA')
            wB = cpool.tile([128, 3 * QKW], BF16, tag='wB')
            nc.vector.memset(wA[:, :], 0.0)
            nc.vector.memset(wB[:, :], 0.0)
            wv_sb = cpool.tile([128, 3 * DH], BF16, tag='wv')
            for c, kk in enumerate(KCH):
                o = sum(KCH[:c])
                nc.sync.dma_start(wA[0:kk, c * QKW:c * QKW + DH],
                                  wq_d[o:o + kk, :])
                nc.sync.dma_start(wA[0:kk, c * QKW + 64:c * QKW + QKW],
                                  wk_d[o:o + kk, :])
                nc.sync.dma_start(wB[0:kk, c * QKW:c * QKW + DH],
                                  wk_d[o:o + kk, :])
                nc.sync.dma_start(wB[0:kk, c * QKW + 64:c * QKW + QKW],
                                  wq_d[o:o + kk, :])
                nc.sync.dma_start(wv_sb[0:kk, c * DH:(c + 1) * DH],
                                  wv_d[o:o + kk, :])
            woT_sb = cpool.tile([DH, D], BF16, tag='woT')
            nc.sync.dma_start(woT_sb[:, :], woT_d)

            xt0 = big.tile([128, s], BF16, tag='xt0')
            xt1 = big.tile([128, s], BF16, tag='xt1')
            xt2 = big.tile([64, s], BF16, tag='xt2')
            xts = (xt0, xt1, xt2)
            # chunked loads so phase 1 starts before the whole xT lands
            for c in range(n_chunks):
                cs = slice(c * CHUNK, (c + 1) * CHUNK)
                nc.sync.dma_start(xt0[:, cs], xT_d[0:128, cs])
                nc.sync.dma_start(xt1[:, cs], xT_d[128:256, cs])
                nc.sync.dma_start(xt2[:, cs], xT_d[256:320, cs])


            qkA = big.tile([QKW, s], BF16, tag='qkA')
            qkB = big.tile([QKW, s], BF16, tag='qkB')
            # v plus zero rows 40..63 and ones row 64: transposing yields v'
            # tiles whose col 64 is 1.0 (the row-sum column).
            vT = big.tile([VW, s], F32, tag='vT')
            vsb = big.tile([128, jt * VW], BF16, tag='vsb')
            oT = big.tile([DH, s], BF16, tag='oT')

            # zero rows 32..63 first (32-aligned base); phase 1 then
            # overwrites rows 0..39 with v, leaving 40..63 zero
            nc.vector.memset(vT[32:VW - 1, :], 0.0)
            nc.vector.memset(vT[VW - 1:VW, :], 1.0)

            # ones at partition 64 for the rec broadcast matmul (fp32)
            ones64 = cpool.tile([65, DH], F32, tag='ones64')
            nc.vector.memset(ones64[64:65, :], 1.0)

            # ---- phase 1 helpers ----
            def proj(dst, w_sb, c, ww):
                ps = ps_small.tile([QKW, CHUNK], F32, tag='small')
                for ci, kk in enumerate(KCH):
                    nc.tensor.matmul(
                        ps[0:ww, :],
                        w_sb[0:kk, ci * ww:(ci + 1) * ww],
                        xts[ci][0:kk, c * CHUNK:(c + 1) * CHUNK],
                        start=(ci == 0), stop=(ci == 2))
                nc.vector.tensor_copy(dst[:, c * CHUNK:(c + 1) * CHUNK],
                                      ps[0:ww, :])

            def transpose_v(j):
                tp = ps_small.tile([128, VW], F32, tag='small')
                nc.tensor.transpose(tp[:, :], vT[:, j * 128:(j + 1) * 128],
                                    ident[0:VW, 0:VW])
                nc.vector.tensor_copy(vsb[:, j * VW:(j + 1) * VW], tp[:, :])

            # qkB (k@0) fully first plus qkA(0): chunk 0 runs its ST
            # unpacked from base-0 operands so exp starts early, while the
            # remaining qkA chunks (k@64 for the packed pairs) project in
            # the background during chunk 0
            for c in range(n_chunks):
                proj(qkB, wB, c, QKW)
            proj(qkA, wA, 0, QKW)

            # ---- main loop over i-chunks ----
            for c in range(n_chunks):
                pts = []
                cs = slice(c * CHUNK, (c + 1) * CHUNK)
                for g in range(jt // GJ):
                    st = ps_st.tile([128, GJ * CHUNK], F32, tag='st2')
                    j0, j1 = g * GJ, g * GJ + 1
                    # chunk 0: both matmuls from base-0 operands (qkA k@64
                    # not projected yet); later chunks: row groups 0 and 64
                    # run concurrently
                    nc.tensor.matmul(
                        st[:, 0:CHUNK],
                        qkB[0:DH, j0 * 128:(j0 + 1) * 128], qkA[0:DH, cs],
                        start=True, stop=True)
                    if c == 0:
                        nc.tensor.matmul(
                            st[:, CHUNK:2 * CHUNK],
                            qkB[0:DH, j1 * 128:(j1 + 1) * 128], qkA[0:DH, cs],
                            start=True, stop=True)
                    else:
                        nc.tensor.matmul(
                            st[:, CHUNK:2 * CHUNK],
                            qkA[64:QKW, j1 * 128:(j1 + 1) * 128],
                            qkB[64:QKW, cs],
                            start=True, stop=True)
                    pt = ptp.tile([128, GJ * CHUNK], BF16, tag='pt')
                    nc.scalar.activation(pt[:, :], st[:, :], EXP, scale=SCALE)
                    pts.append(pt)

                if c == 0:
                    for cc in range(1, n_chunks):
                        proj(qkA, wA, cc, QKW)
                    for cc in range(n_chunks):
                        proj(vT[0:DH, :], wv_sb, cc, DH)
                    for j in range(jt):
                        transpose_v(j)

                avq = [ps_av.tile([VW, CHUNK], F32, tag=f'av{q}',
                                  name=f'av{q}')
                       for q in range(2)]
                for j in range(jt):
                    ptt = pts[j // GJ]
                    pcs = slice((j % GJ) * CHUNK, (j % GJ + 1) * CHUNK)
                    for q in range(2):
                        nc.tensor.matmul(
                            avq[q][:, :],
                            vsb[64 * q:64 * (q + 1), j * VW:(j + 1) * VW],
                            ptt[64 * q:64 * (q + 1), pcs],
                            start=(j == 0), stop=(j == jt - 1),
                            tile_position=(64 * q, 0))

                # merge the 2 partial accumulators on DVE
                m1 = wkp.tile([VW, CHUNK], F32, tag='m1')
                nc.vector.tensor_copy(m1[:, :], avq[0][:, :])
                nc.vector.tensor_tensor(
                    out=m1[:, :], in0=avq[1][:, :], in1=m1[:, :],
                    op=mybir.AluOpType.add)

                rec = wkp.tile([65, CHUNK], F32, tag='rec')
                nc.vector.reciprocal(rec[64:65, :], m1[64:65, :])
                rbc = ps_small.tile([DH, CHUNK], F32, tag='small')
                nc.tensor.matmul(rbc[:, :], ones64[64:65, :],
                                 rec[64:65, :], start=True, stop=True)
                rbc_sb = wkp.tile([DH, CHUNK], F32, tag='rbc_sb')
                nc.vector.tensor_copy(rbc_sb[:, :], rbc[:, :])
                nc.vector.tensor_tensor(
                    out=oT[:, c * CHUNK:(c + 1) * CHUNK],
                    in0=m1[0:DH, :], in1=rbc_sb[:, :],
                    op=mybir.AluOpType.mult)

                # output projection for the 4 s-tiles of this chunk
                for s2 in range(CHUNK // 128):
                    st_i = c * (CHUNK // 128) + s2
                    yp = ps_small.tile([128, D], F32, tag='small')
                    nc.tensor.matmul(yp[:, :],
                                     oT[:, st_i * 128:(st_i + 1) * 128],
                                     woT_sb[:, :], start=True, stop=True)
                    ysb = wkp.tile([128, D], F32, tag='ysb')
                    nc.vector.tensor_copy(ysb[:, :], yp[:, :])
                    nc.sync.dma_start(out_d[st_i * 128:(st_i + 1) * 128, :],
                                      ysb[:, :])

    if split:
        _split_sync_waits(nc)
    return nc


def kernel(x, Wq, Wk, Wv, Wo, bo):
    x = np.asarray(x, dtype=np.float32)
    Wq = np.asarray(Wq, dtype=np.float32)
    Wk = np.asarray(Wk, dtype=np.float32)
    Wv = np.asarray(Wv, dtype=np.float32)
    Wo = np.asarray(Wo, dtype=np.float32)
    bo = np.asarray(bo, dtype=np.float32)

    if 'nc' not in _COMPILED:
        _COMPILED['nc'] = _build()
    nc = _COMPILED['nc']

    bf = ml_dtypes.bfloat16
    xT = np.ascontiguousarray(x.reshape(S, D).T).astype(bf)
    in_maps = []
    for h in range(N_CORES):
        sl = slice(h * DH, (h + 1) * DH)
        in_maps.append({
            'xT': xT,
            'wq': np.ascontiguousarray(Wq[sl, :].T).astype(bf),
            'wk': np.ascontiguousarray(Wk[sl, :].T).astype(bf),
            'wv': np.ascontiguousarray(Wv[sl, :].T).astype(bf),
            'woT': np.ascontiguousarray(Wo[:, sl].T).astype(bf),
        })

    trace = bool(os.environ.get('BASS_KERNEL_TRACE'))

    def _run():
        return bass_utils.run_bass_kernel_spmd(
            nc, in_maps, core_ids=list(range(N_CORES)), trace=trace,
            tmpdir=os.environ.get('BASS_KERNEL_TRACE_DIR') or None)

    try:
        res = _run()
    except Exception:
        # A previously crashed NEFF can leave the device unrecoverable; the
        # failed attempt clears it, so one retry is usually enough.
        res = _run()
    _COMPILED['last_res'] = res

    acc = res.results[0]['out'].astype(np.float32).copy()
    for h in range(1, N_CORES):
        acc += res.results[h]['out']
    acc += bo[None, :]
    return acc.reshape(1, S, D)
